# revision 3
# baseline (speedup 1.0000x reference)
"""Trainium2 Bass kernel for nn_MetaUpSample (2x meta-upsample, 3x3 dynamic filters).

out[b,ho,wo,f] = sum_k patches[b,ho,wo,k] * meta_w[b,ho,wo,k*3+f]
  patches[b,ho,wo,(dk0,dk1,c)] = x_pad[b, ho//2+dk0, wo//2+dk1, c]

Sharding: 8 cores, core ci handles b = ci//2, ho in [(ci%2)*64, (ci%2)*64+64).
meta_w (432 MiB total) is the dominant HBM stream (~56.6 MiB/core); the kernel
streams it once and fuses multiply+reduce in single scalar_tensor_tensor ops
(out = (mw * 1.0) * patch, accum_out = per-partition sum over K), split 5:1
between the Vector and GpSimd engines.

Host side pre-builds the duplicated patch-row tensor xrb (x is only 4 MiB) and
un-transposes the [wo, (ho,f)] device output, so the device graph is just:
  1 xrb DMA + 32 x (1.77MiB meta_w DMA + 6 fused multiply-reduce ops) + 1 out DMA.
meta_w DMAs alternate between the SP and ACT HWDGE rings.
"""
from contextlib import ExitStack

import numpy as np

import concourse.bass as bass
import concourse.mybir as mybir
from concourse.bass_utils import run_bass_kernel_spmd

B, H, W, C = 4, 64, 64, 64
HO, WO, F = 128, 128, 3
KS = 3
K = KS * KS * C            # 576
QF = K * F                 # 1728 meta_w channels
RW = KS * C                # 192 floats per patch row (dk1, c)
N_CORES = 8
CORES_PER_B = N_CORES // B         # 2
HO_PC = HO // CORES_PER_B          # 64 output rows per core
NHS = HO_PC // 2                   # 32 hs tiles per core
NROWS = NHS + 2                    # 34 cached padded x rows per core

NBUF = 7     # meta_w double-buffer slots
NSCR = 4     # DVE scratch ring (same-engine WAW spacing)
NTG = 3      # gpsimd->ACT product-buffer ring
NSCRA = 2    # ACT copy-out scratch ring

f32 = mybir.dt.float32

# op assignment within a tile: (r, f) of the op routed GpSimd(mult)+ACT(reduce)
GP_R, GP_F = 1, 2
V_PER_TILE = 5

_CACHED = None


def _build_nc():
    nc = bass.Bass()
    mw_d = nc.declare_dram_parameter("mw", [HO_PC, WO, QF], f32, isOutput=False)
    xrb_d = nc.declare_dram_parameter("xrb", [WO, NROWS * RW], f32, isOutput=False)
    out_d = nc.declare_dram_parameter("out", [WO, HO_PC * F], f32, isOutput=True)

    with ExitStack() as ctx:
        xrow = ctx.enter_context(nc.sbuf_tensor([WO, NROWS * RW], f32))
        mwbuf = ctx.enter_context(nc.sbuf_tensor([WO, NBUF * 2 * QF], f32))
        scr_v = ctx.enter_context(nc.sbuf_tensor([WO, NSCR * K], f32))
        tg = ctx.enter_context(nc.sbuf_tensor([WO, NTG * K], f32))
        scr_a = ctx.enter_context(nc.sbuf_tensor([WO, NSCRA * K], f32))
        out_sb = ctx.enter_context(nc.sbuf_tensor([WO, HO_PC * F], f32))
        slot_sem = [ctx.enter_context(nc.semaphore(f"slot{j}")) for j in range(NBUF)]
        misc_sem = ctx.enter_context(nc.semaphore("misc"))
        cmp_v = ctx.enter_context(nc.semaphore("cmp_v"))   # DVE fused ops done
        gmul = ctx.enter_context(nc.semaphore("gmul"))     # gpsimd products done
        cmp_g = ctx.enter_context(nc.semaphore("cmp_g"))   # ACT reduces done
        block = ctx.enter_context(nc.Block())

        def slot_ap(j):
            return mwbuf[:, j * 2 * QF : (j + 1) * 2 * QF]

        def mw4(j):
            return slot_ap(j).rearrange("p (h k f) -> p h k f", h=2, f=F)

        def win(i):
            return xrow[:, i * RW : i * RW + KS * RW]

        @block.sync
        def _(sync):
            for i in range(NHS):
                j, p = i % NBUF, i // NBUF
                if p > 0:
                    # compute engines finished reading the slot's previous tile
                    sync.wait_ge(cmp_v, V_PER_TILE * (i - NBUF + 1))
                    sync.wait_ge(gmul, i - NBUF + 1)
                    # provably satisfied; makes the inc ordering explicit
                    sync.wait_ge(slot_sem[j], 16 * p)
                sync.dma_start(
                    out=slot_ap(j).rearrange("p (h q) -> p h q", h=2),
                    in_=mw_d[2 * i : 2 * i + 2].rearrange("h w q -> w h q"),
                ).then_inc(slot_sem[j], 16)
            sync.wait_ge(cmp_v, NHS * V_PER_TILE)
            sync.wait_ge(cmp_g, NHS)
            sync.dma_start(out=out_d[:], in_=out_sb[:]).then_inc(misc_sem, 16)

        @block.vector
        def _(vector):
            vector.wait_ge(misc_sem, 16)
            nv = 0
            for i in range(NHS):
                j, p = i % NBUF, i // NBUF
                vector.wait_ge(slot_sem[j], 16 * (p + 1))
                for r in range(2):
                    for f in range(F):
                        if (r, f) == (GP_R, GP_F):
                            continue
                        if nv >= NSCR:
                            # same-engine WAW spacing on the scratch ring;
                            # already satisfied at runtime (DVE incs cmp_v)
                            vector.wait_ge(cmp_v, nv - NSCR + 1)
                        vector.scalar_tensor_tensor(
                            out=scr_v[:, (nv % NSCR) * K : (nv % NSCR + 1) * K],
                            in0=mw4(j)[:, r, :, f],
                            scalar=1.0,
                            in1=win(i),
                            op0=mybir.AluOpType.mult,
                            op1=mybir.AluOpType.mult,
                            accum_out=out_sb[:, (2 * i + r) * F + f : (2 * i + r) * F + f + 1],
                        ).then_inc(cmp_v, 1)
                        nv += 1

        @block.gpsimd
        def _(gpsimd):
            gpsimd.wait_ge(misc_sem, 16)
            for i in range(NHS):
                j, p = i % NBUF, i // NBUF
                gpsimd.wait_ge(slot_sem[j], 16 * (p + 1))
                if i >= NTG:
                    # ACT finished reading this tg ring slot's previous product
                    gpsimd.wait_ge(cmp_g, i - NTG + 1)
                gpsimd.tensor_tensor(
                    tg[:, (i % NTG) * K : (i % NTG + 1) * K],
                    mw4(j)[:, GP_R, :, GP_F],
                    win(i),
                    mybir.AluOpType.mult,
                ).then_inc(gmul, 1)

        @block.scalar
        def _(scalar):
            scalar.dma_start(out=xrow[:], in_=xrb_d[:]).then_inc(misc_sem, 16)
            for i in range(NHS):
                scalar.wait_ge(gmul, i + 1)
                if i >= NSCRA:
                    # own-engine WAW spacing on the copy-out ring
                    scalar.wait_ge(cmp_g, i - NSCRA + 1)
                scalar.activation(
                    scr_a[:, (i % NSCRA) * K : (i % NSCRA + 1) * K],
                    tg[:, (i % NTG) * K : (i % NTG + 1) * K],
                    mybir.ActivationFunctionType.Copy,
                    accum_out=out_sb[
                        :, (2 * i + GP_R) * F + GP_F : (2 * i + GP_R) * F + GP_F + 1
                    ],
                ).then_inc(cmp_g, 1)

    return nc


def _prep_xrb(x):
    """Per-core duplicated patch-row tensors.

    xrb[ci][wo, hpl*RW + dk1*C + c] = x_pad[b, hs0+hpl, wo//2 + dk1, c]
    where x_pad has 1 zero row/col of padding on each side.
    """
    from numpy.lib.stride_tricks import sliding_window_view

    out = []
    for ci in range(N_CORES):
        b, hs0 = ci // CORES_PER_B, (ci % CORES_PER_B) * NHS
        xp = np.pad(x[b], ((1, 1), (1, 1), (0, 0)))          # [66, 66, 64]
        rows = xp[hs0 : hs0 + NROWS]                          # [34, 66, 64]
        win = sliding_window_view(rows, KS, axis=1)           # [34, 64(ws), 64(c), 3(dk1)]
        win = win.transpose(0, 1, 3, 2).reshape(NROWS, W, RW)  # [34, 64, 192]
        dup = np.repeat(win, 2, axis=1)                       # [34, 128, 192]
        out.append(
            np.ascontiguousarray(dup.transpose(1, 0, 2)).reshape(WO, NROWS * RW)
        )
    return out


last_results = None  # BassKernelResults of the most recent kernel() call


def kernel(x, meta_w):
    global _CACHED, last_results
    x = np.ascontiguousarray(np.asarray(x, dtype=np.float32))
    meta_w = np.asarray(meta_w, dtype=np.float32)

    if _CACHED is None:
        _CACHED = _build_nc()
    nc = _CACHED

    xrbs = _prep_xrb(x)
    in_maps = []
    for ci in range(N_CORES):
        b, ho0 = ci // CORES_PER_B, (ci % CORES_PER_B) * HO_PC
        in_maps.append(
            {"mw": meta_w[b, ho0 : ho0 + HO_PC], "xrb": xrbs[ci]}
        )

    res = run_bass_kernel_spmd(nc, in_maps, list(range(N_CORES)))
    last_results = res

    out = np.empty((B, HO, WO, F), np.float32)
    for ci in range(N_CORES):
        b, ho0 = ci // CORES_PER_B, (ci % CORES_PER_B) * HO_PC
        o = res.results[ci]["out"].reshape(WO, HO_PC, F)
        out[b, ho0 : ho0 + HO_PC] = o.transpose(1, 0, 2)
    return out
